# revision 63
# baseline (speedup 1.0000x reference)
"""Linformer self-attention (degenerate-einsum variant) on 8 TRN2 NeuronCores.

Math (from the reference):
  k_proj[b,h,k,d] = E[k,d] * S_k[b,h*64+d]  where S_k[b,:] = (sum_n x[b,n,:]) @ Wk.T
  (the einsum 'bhnd,kd->bhkd' sums k over n, elementwise in d; the sequence sum
   commutes with the linear projection, so k/v never need materializing)
  attn = softmax( (q * S_k) @ E.T / 8 )  per (b, head)
  out  = (attn @ (F * S_v)) restored to (B,N,D), then @ Wo.T + bo

Sharding: core c = (batch b = c//2, sequence half = c%2); each core computes a
(2048, 1024) slice of the output. Host precomputes S_k/S_v (tiny) and folds
them into per-head E-hat / F-hat (block-diagonal pair packing), pre-transposes
x / Wq / Wo. All matmul operands are fp16 (PE streams fp16 at the same
1 col/cycle as fp32r, but DMA/SBUF halve); accumulation stays fp32 in PSUM.

The (n,k)->(k,n) attention transposes ride the DMA XBAR (dma_start_transpose,
2-byte dtype, per-128-col-block transpose semantics) instead of the PE; they
are issued one stage after production and consumed two half-blocks later, so
neither the in-order DMA queue nor the PE ever waits on them. The softmax
logits/q operands stay fp32r (argmax-sharp softmax needs ~11 mantissa bits);
softmax sum/normalize ride the DVE 2x 16-bit path; output stores are fp16
(upcast on host) via the Activation HWDGE queue.
"""

import numpy as np
import ml_dtypes

import concourse.bass as bass
import concourse.bacc as bacc
import concourse.tile as tile
import concourse.mybir as mybir
import concourse.bass_utils as bass_utils

B, N, D = 4, 4096, 1024
H, HD, KP = 16, 64, 256  # heads, head dim, linformer K
NCORES = 8
NH = N // 2          # rows per core = 2048
HBLK = 256           # half-block rows
NHB = NH // HBLK     # 8 half-blocks
F32 = mybir.dt.float32
F32R = mybir.dt.float32r
F16 = mybir.dt.float16
BF16 = mybir.dt.bfloat16

_CACHE = {}


def _round_fp32r(a: np.ndarray) -> np.ndarray:
    """Round-to-nearest-even fp32 -> fp32r (11 explicit mantissa bits)."""
    b = np.ascontiguousarray(a, dtype=np.float32).view(np.uint32)
    low = b & np.uint32(0xFFF)
    bit12 = (b >> np.uint32(12)) & np.uint32(1)
    up = (low > 0x800) | ((low == 0x800) & (bit12 == 1))
    r = (b & np.uint32(0xFFFFF000)) + (up.astype(np.uint32) << np.uint32(12))
    return r.view(np.float32)


def _build():
    nc = bacc.Bacc("TRN2", target_bir_lowering=False, debug=False, num_devices=NCORES)

    xT_d = nc.dram_tensor("xT", [D, NH], F16, kind="ExternalInput").ap()
    wqT_d = nc.dram_tensor("wqT", [D, D], F16, kind="ExternalInput").ap()
    woT_d = nc.dram_tensor("woT", [D, D], F16, kind="ExternalInput").ap()
    ehat_d = nc.dram_tensor("ehat", [128, 8, 2 * KP], F32R, kind="ExternalInput").ap()
    fhat_d = nc.dram_tensor("fhat", [128, 8, 2, 2, 128], F16, kind="ExternalInput").ap()
    out_d = nc.dram_tensor("out", [NH, D], F16, kind="ExternalOutput").ap()

    with tile.TileContext(nc) as tc:
        with (
            tc.tile_pool(name="wq", bufs=1) as wq_pool,
            tc.tile_pool(name="wo", bufs=1) as wo_pool,
            tc.tile_pool(name="const", bufs=1) as const_pool,
            tc.tile_pool(name="xt", bufs=25) as xt_pool,
            tc.tile_pool(name="qt", bufs=12) as qt_pool,
            tc.tile_pool(name="estat", bufs=8) as stat_pool,
            tc.tile_pool(name="eg", bufs=6) as e_pool,
            tc.tile_pool(name="pt", bufs=7) as pt_pool,
            tc.tile_pool(name="ohat", bufs=10) as ohat_pool,
            tc.tile_pool(name="osb", bufs=3) as out_pool,
            tc.tile_pool(name="qfpsum", bufs=2, space=bass.MemorySpace.PSUM) as qfpsum,
            tc.tile_pool(name="apsum", bufs=4, space=bass.MemorySpace.PSUM) as apsum,
            tc.tile_pool(name="opsum", bufs=2, space=bass.MemorySpace.PSUM) as opsum,
        ):
            # ---- block-0 activations first: unblocks the first Q matmuls ----
            xt_state = {}

            def load_x(blk):
                xt = []
                for c in range(8):
                    t = xt_pool.tile([128, 512], F16, tag="xt", name=f"xt{c}")
                    nc.sync.dma_start(
                        t[:], xT_d[c * 128:(c + 1) * 128, blk * 512:(blk + 1) * 512]
                    )
                    xt.append(t)
                xt_state[blk] = xt

            # ---- persistent weights (wq/ehat first: needed immediately;
            # wo/fhat only needed once stage_b starts) ----
            wq_sb = []
            wo_sb = []
            for c in range(8):
                t = wq_pool.tile([128, D], F16, tag=f"wq{c}")
                wq_sb.append(t)
            # interleave block-0 x tiles with wq so the first Q matmuls
            # (which consume (xt[ck], wq[ck]) in ck order) start ~2 DMAs in
            xt0 = []
            for c in range(8):
                t = xt_pool.tile([128, 512], F16, tag="xt", name=f"xt{c}")
                xt0.append(t)
            xt_state[0] = xt0
            for c in range(8):
                nc.sync.dma_start(xt0[c][:], xT_d[c * 128:(c + 1) * 128, 0:512])
                nc.sync.dma_start(wq_sb[c][:], wqT_d[c * 128:(c + 1) * 128, :])
            ehat_sb = const_pool.tile([128, 8, 2 * KP], F32R, tag="ehat")
            nc.sync.dma_start(ehat_sb[:], ehat_d[:])
            for c in range(8):
                t = wo_pool.tile([128, D], F16, tag=f"wo{c}")
                nc.sync.dma_start(t[:], woT_d[c * 128:(c + 1) * 128, :])
                wo_sb.append(t)
            fhat_sb = const_pool.tile([128, 8, 2, 2, 128], F16, tag="fhat")
            nc.sync.dma_start(fhat_sb[:], fhat_d[:])

            # ---- software-pipelined main loop over half-blocks of 256 rows ----
            # stage A(hb): DMA xT, Q-proj, attn logits + softmax -> pT via XBAR
            # stage B(hb): ohat, final, store — emitted one hb late so the PE
            # never waits on freshly-computed softmax results.
            p_state = {}
            pts_state = {}

            pending_xposes = []

            def flush_xposes(cur_idx=None):
                # XBAR: per-128-col-block transpose of (n, [hh,k]) ->
                # [k-part, (hh, kchunk), n] written into the s half.
                # Each batch is issued one full stage after production (age
                # >= 2 half-stages) so the dependency (normalize) is long
                # satisfied -> the in-order DMA queue never spins on it,
                # keeping x-loads behind it on schedule.
                while pending_xposes and (
                    cur_idx is None or pending_xposes[0][0] <= cur_idx - 2
                ):
                    _, ptsg, s, e_g = pending_xposes.pop(0)
                    nc.sync.dma_start_transpose(ptsg[:, :, :, s, :], e_g[:])

            def stage_a(hb, s_range):
                blk = hb // 2
                if 0 in s_range:
                    # prefetch next block's x tiles first: dep-free loads at
                    # the head of the DMA queue, ~2 stages before consumption
                    nblk = blk + (hb % 2)
                    if nblk + 1 < NHB // 2 and (nblk + 1, "qt") not in p_state:
                        if nblk + 1 not in xt_state:
                            load_x(nblk + 1)
                flush_xposes(2 * hb + s_range[0])

                def q_chunks(b, cos):
                    # compute Q chunks `cos` of block b
                    if b not in xt_state:
                        load_x(b)
                    xt = xt_state[b]
                    qt = p_state.setdefault((b, "qt"), {})
                    for co in cos:
                        qp = qfpsum.tile([128, 512], F32, tag="qf", name=f"qp{co}")
                        for ck in range(8):
                            nc.tensor.matmul(
                                qp[:],
                                wq_sb[ck][:, co * 128:(co + 1) * 128],
                                xt[ck][:],
                                start=(ck == 0),
                                stop=(ck == 7),
                            )
                        q_sb = qt_pool.tile([128, 512], F32R, tag="qt", name=f"q{co}")
                        nc.scalar.copy(q_sb[:], qp[:])
                        qt[co] = q_sb
                    if max(cos) == 7:
                        xt_state.pop(b, None)

                if 0 in s_range:
                    if hb == 0:
                        q_chunks(0, range(8))
                    elif hb % 2 == 0:
                        q_chunks(blk, range(4, 8))
                    else:
                        if blk + 1 < NHB // 2:
                            q_chunks(blk + 1, range(0, 4))
                qt = p_state[(blk, "qt")]

                for s in s_range:
                    sb = (hb % 2) * 2 + s
                    for gp in range(2):  # group-pairs of 4 pairs = 8 heads
                        aps = []
                        negmax = stat_pool.tile([128, 8], F32, tag="negmax")
                        ssum = stat_pool.tile([128, 8], F16, tag="ssum")
                        for jj in range(4):
                            j = 4 * gp + jj
                            ap_ = apsum.tile([128, 2 * KP], F32, tag="ap", name=f"ap{j}")
                            nc.tensor.matmul(
                                ap_[:],
                                qt[j][:, sb * 128:(sb + 1) * 128],
                                ehat_sb[:, j, :],
                                start=True,
                                stop=True,
                            )
                            aps.append(ap_)
                            nc.vector.reduce_max(
                                negmax[:, 2 * jj:2 * jj + 2],
                                ap_[:].rearrange("p (c k) -> p c k", c=2),
                                axis=mybir.AxisListType.X, negate=True,
                            )
                        e_g = e_pool.tile([128, 8, KP], F16, tag="eg", name=f"eg{gp}")
                        for hh in range(8):
                            nc.scalar.activation(
                                e_g[:, hh, :],
                                aps[hh // 2][:, (hh % 2) * KP:(hh % 2 + 1) * KP],
                                mybir.ActivationFunctionType.Exp,
                                bias=negmax[:, hh:hh + 1],
                            )
                        # fp16 ssum store keeps the reduce + normalize on the
                        # DVE 2x 16-bit path; the reduce accumulates
                        # internally at full precision, only the store rounds
                        with nc.allow_low_precision(reason="fp16 softmax-sum store"):
                            nc.vector.reduce_sum(
                                ssum[:], e_g[:], axis=mybir.AxisListType.X
                            )
                        recip = stat_pool.tile([128, 8], F32, tag="recip")
                        nc.vector.reciprocal(recip[:], ssum[:])
                        # duplicated fp16 recip: gives the broadcast operand a
                        # packed stride-1 innermost dim (DVE 2x requirement)
                        recip16 = stat_pool.tile([128, 8, 2], F16, tag="recip16")
                        nc.vector.tensor_copy(
                            recip16[:], recip[:].unsqueeze(2).broadcast_to((128, 8, 2))
                        )
                        nc.vector.tensor_tensor(
                            e_g[:].rearrange("p h (kk kl) -> p h kk kl", kl=2),
                            e_g[:].rearrange("p h (kk kl) -> p h kk kl", kl=2),
                            recip16[:].unsqueeze(2).broadcast_to((128, 8, KP // 2, 2)),
                            op=mybir.AluOpType.mult,
                        )
                        if s == 0:
                            ptsg = pt_pool.tile(
                                [128, 8, 2, 2, 128], F16, tag="ptsg", name=f"pts{gp}"
                            )
                            pts_state[(hb, gp)] = ptsg
                        else:
                            ptsg = pts_state[(hb, gp)]
                        pending_xposes.append((2 * hb + s, ptsg, s, e_g))

            def stage_b(hb):
                r0 = hb * HBLK
                ptsg = [pts_state.pop((hb, gp)) for gp in range(2)]
                ohatT = []
                for j in range(8):
                    op_ = opsum.tile([128, HBLK], F32, tag="op", name=f"op{j}")
                    first = True
                    for hh2 in range(2):
                        gp, hh = j // 4, (j % 4) * 2
                        for c in range(2):
                            nc.tensor.matmul(
                                op_[:],
                                fhat_sb[:, j, hh2, c, :],
                                ptsg[gp][:, hh + hh2, c, :, :],
                                start=first,
                                stop=(hh2 == 1 and c == 1),
                            )
                            first = False
                    oT = ohat_pool.tile([128, HBLK], F16, tag="ohatT", name=f"oT{j}")
                    nc.scalar.copy(oT[:], op_[:])
                    ohatT.append(oT)
                for s in range(2):
                    for half in range(2):
                        fp_ = qfpsum.tile([128, 512], F32, tag="qf", name=f"fp{s}{half}")
                        for j in range(8):
                            nc.tensor.matmul(
                                fp_[:],
                                ohatT[j][:, s * 128:(s + 1) * 128],
                                wo_sb[j][:, half * 512:(half + 1) * 512],
                                start=(j == 0),
                                stop=(j == 7),
                            )
                        o_sb = out_pool.tile([128, 512], F16, tag="osb", name=f"o{s}{half}")
                        nc.scalar.copy(o_sb[:], fp_[:])
                        # store via the Activation HWDGE queue: keeps the sync
                        # queue spin-free for x-loads + XBAR transposes
                        nc.scalar.dma_start(
                            out_d[r0 + s * 128:r0 + (s + 1) * 128,
                                  half * 512:(half + 1) * 512],
                            o_sb[:],
                        )

            # stage_b consumes the XBAR-transposed attention two half-blocks
            # late (DMA transpose latency never on the PE's path), and is
            # emitted between the s=0 and s=1 softmax halves so the PE chews
            # stage_b matmuls while the s=0 softmax chain drains on
            # scalar/DVE (frees the logits PSUM banks for s=1)
            for hb in range(NHB + 2):
                if hb < NHB:
                    stage_a(hb, (0,))
                else:
                    flush_xposes()
                if hb >= 2:
                    stage_b(hb - 2)
                if hb < NHB:
                    stage_a(hb, (1,))
                    if hb % 2 == 1:
                        p_state.pop((hb // 2, "qt"), None)

    nc.compile()
    return nc


def _prep_inputs(x, Wq, Wk, Wv, E, F, Wo, bo):
    x = np.asarray(x, dtype=np.float32)
    Wq = np.asarray(Wq, dtype=np.float32)
    Wk = np.asarray(Wk, dtype=np.float32)
    Wv = np.asarray(Wv, dtype=np.float32)
    E = np.asarray(E, dtype=np.float32)
    F_ = np.asarray(F, dtype=np.float32)
    Wo = np.asarray(Wo, dtype=np.float32)
    bo = np.asarray(bo, dtype=np.float32)

    xsum = x.sum(axis=1)                     # (B, D)
    S_k = xsum @ Wk.T                        # (B, D)
    S_v = xsum @ Wv.T                        # (B, D)

    wqT = np.ascontiguousarray(Wq.T).astype(np.float16)
    woT = np.ascontiguousarray(Wo.T).astype(np.float16)

    in_maps = []
    for core in range(NCORES):
        b, half = core // 2, core % 2
        xs = x[b, half * NH:(half + 1) * NH, :]          # (NH, D)
        xT = np.ascontiguousarray(xs.T).astype(np.float16)  # (D, NH)

        # E-hat: block-diagonal per head pair -> one (128,512) rhs per pair
        ehat = np.zeros((128, 8, 2 * KP), dtype=np.float32)
        for h in range(H):
            sk = S_k[b, h * HD:(h + 1) * HD]             # (64,)
            j, hh = h // 2, h % 2
            ehat[hh * 64:hh * 64 + 64, j, hh * KP:(hh + 1) * KP] = (E.T * sk[:, None]) / 8.0
        ehat = _round_fp32r(ehat)

        # F-hat: block-diagonal pair packing, (128, pair, head-in-pair, chunk, 64*2)
        fhat = np.zeros((128, 8, 2, 2, 128), dtype=np.float32)
        for h in range(H):
            sv = S_v[b, h * HD:(h + 1) * HD]             # (64,)
            fh = F_ * sv[None, :]                        # (KP, 64)
            j, hh = h // 2, h % 2
            for c in range(2):
                fhat[:, j, hh, c, hh * 64:(hh + 1) * 64] = fh[c * 128:(c + 1) * 128, :]
        fhat = fhat.astype(np.float16)

        in_maps.append({
            "xT": xT, "wqT": wqT, "woT": woT, "ehat": ehat,
            "fhat": fhat,
        })
    return in_maps, bo.reshape(1, 1, D)


def _run(inputs: dict, trace: bool = False, tmpdir: str | None = None):
    if "nc" not in _CACHE:
        _CACHE["nc"] = _build()
    nc = _CACHE["nc"]
    in_maps, bo_row = _prep_inputs(**inputs)
    res = bass_utils.run_bass_kernel_spmd(
        nc, in_maps, core_ids=list(range(NCORES)), trace=trace, tmpdir=tmpdir
    )
    out = np.empty((B, N, D), dtype=np.float32)
    for core in range(NCORES):
        b, half = core // 2, core % 2
        out[b, half * NH:(half + 1) * NH, :] = res.results[core]["out"]
    out += bo_row  # bias rides the host epilogue, not the device
    return out, res


def kernel(**inputs) -> np.ndarray:
    out, _ = _run(inputs)
    return out


# revision 64
# speedup vs baseline: 1.0988x; 1.0988x over previous
"""Linformer self-attention (degenerate-einsum variant) on 8 TRN2 NeuronCores.

Math (from the reference):
  k_proj[b,h,k,d] = E[k,d] * S_k[b,h*64+d]  where S_k[b,:] = (sum_n x[b,n,:]) @ Wk.T
  (the einsum 'bhnd,kd->bhkd' sums k over n, elementwise in d; the sequence sum
   commutes with the linear projection, so k/v never need materializing)
  attn = softmax( (q * S_k) @ E.T / 8 )  per (b, head)
  out  = (attn @ (F * S_v)) restored to (B,N,D), then @ Wo.T + bo

Sharding: core c = (batch b = c//2, sequence half = c%2); each core computes a
(2048, 1024) slice of the output. Host precomputes S_k/S_v (tiny) and folds
them into per-head E-hat / F-hat (block-diagonal pair packing), pre-transposes
x / Wq / Wo. All matmul operands are fp16 (PE streams fp16 at the same
1 col/cycle as fp32r, but DMA/SBUF halve); accumulation stays fp32 in PSUM.

The (n,k)->(k,n) attention transposes ride the DMA XBAR (dma_start_transpose,
2-byte dtype, per-128-col-block transpose semantics) instead of the PE; they
are issued one stage after production and consumed two half-blocks later, so
neither the in-order DMA queue nor the PE ever waits on them. The softmax
logits/q operands stay fp32r (argmax-sharp softmax needs ~11 mantissa bits);
softmax sum/normalize ride the DVE 2x 16-bit path; output stores are fp16
(upcast on host) via the Activation HWDGE queue.
"""

import numpy as np
import ml_dtypes

import concourse.bass as bass
import concourse.bacc as bacc
import concourse.tile as tile
import concourse.mybir as mybir
import concourse.bass_utils as bass_utils

B, N, D = 4, 4096, 1024
H, HD, KP = 16, 64, 256  # heads, head dim, linformer K
NCORES = 8
NH = N // 2          # rows per core = 2048
HBLK = 256           # half-block rows
NHB = NH // HBLK     # 8 half-blocks
F32 = mybir.dt.float32
F32R = mybir.dt.float32r
F16 = mybir.dt.float16
BF16 = mybir.dt.bfloat16

_CACHE = {}


def _round_fp32r(a: np.ndarray) -> np.ndarray:
    """Round-to-nearest-even fp32 -> fp32r (11 explicit mantissa bits)."""
    b = np.ascontiguousarray(a, dtype=np.float32).view(np.uint32)
    low = b & np.uint32(0xFFF)
    bit12 = (b >> np.uint32(12)) & np.uint32(1)
    up = (low > 0x800) | ((low == 0x800) & (bit12 == 1))
    r = (b & np.uint32(0xFFFFF000)) + (up.astype(np.uint32) << np.uint32(12))
    return r.view(np.float32)


def _build():
    nc = bacc.Bacc("TRN2", target_bir_lowering=False, debug=False, num_devices=NCORES)

    xT_d = nc.dram_tensor("xT", [D, NH], F16, kind="ExternalInput").ap()
    wqT_d = nc.dram_tensor("wqT", [D, D], F16, kind="ExternalInput").ap()
    woT_d = nc.dram_tensor("woT", [D, D], F16, kind="ExternalInput").ap()
    ehat_d = nc.dram_tensor("ehat", [128, 8, 2 * KP], F32R, kind="ExternalInput").ap()
    fhat_d = nc.dram_tensor("fhat", [128, 8, 2, 2, 128], F16, kind="ExternalInput").ap()
    bo_d = nc.dram_tensor("bo", [1, D], F16, kind="ExternalInput").ap()
    ones_d = nc.dram_tensor("ones", [1, 128], F16, kind="ExternalInput").ap()
    out_d = nc.dram_tensor("out", [NH, D], F16, kind="ExternalOutput").ap()

    with tile.TileContext(nc) as tc:
        with (
            tc.tile_pool(name="wq", bufs=1) as wq_pool,
            tc.tile_pool(name="wo", bufs=1) as wo_pool,
            tc.tile_pool(name="const", bufs=1) as const_pool,
            tc.tile_pool(name="xt", bufs=25) as xt_pool,
            tc.tile_pool(name="qt", bufs=12) as qt_pool,
            tc.tile_pool(name="estat", bufs=8) as stat_pool,
            tc.tile_pool(name="eg", bufs=6) as e_pool,
            tc.tile_pool(name="pt", bufs=7) as pt_pool,
            tc.tile_pool(name="ohat", bufs=10) as ohat_pool,
            tc.tile_pool(name="osb", bufs=3) as out_pool,
            tc.tile_pool(name="qfpsum", bufs=2, space=bass.MemorySpace.PSUM) as qfpsum,
            tc.tile_pool(name="apsum", bufs=4, space=bass.MemorySpace.PSUM) as apsum,
            tc.tile_pool(name="opsum", bufs=2, space=bass.MemorySpace.PSUM) as opsum,
        ):
            # ---- block-0 activations first: unblocks the first Q matmuls ----
            xt_state = {}

            def load_x(blk):
                xt = []
                for c in range(8):
                    t = xt_pool.tile([128, 512], F16, tag="xt", name=f"xt{c}")
                    nc.sync.dma_start(
                        t[:], xT_d[c * 128:(c + 1) * 128, blk * 512:(blk + 1) * 512]
                    )
                    xt.append(t)
                xt_state[blk] = xt

            # ---- persistent weights (wq/ehat first: needed immediately;
            # wo/fhat only needed once stage_b starts) ----
            wq_sb = []
            wo_sb = []
            for c in range(8):
                t = wq_pool.tile([128, D], F16, tag=f"wq{c}")
                wq_sb.append(t)
            # interleave block-0 x tiles with wq so the first Q matmuls
            # (which consume (xt[ck], wq[ck]) in ck order) start ~2 DMAs in
            xt0 = []
            for c in range(8):
                t = xt_pool.tile([128, 512], F16, tag="xt", name=f"xt{c}")
                xt0.append(t)
            xt_state[0] = xt0
            for c in range(8):
                nc.sync.dma_start(xt0[c][:], xT_d[c * 128:(c + 1) * 128, 0:512])
                nc.sync.dma_start(wq_sb[c][:], wqT_d[c * 128:(c + 1) * 128, :])
            ehat_sb = const_pool.tile([128, 8, 2 * KP], F32R, tag="ehat")
            nc.sync.dma_start(ehat_sb[:], ehat_d[:])
            for c in range(8):
                t = wo_pool.tile([128, D], F16, tag=f"wo{c}")
                nc.sync.dma_start(t[:], woT_d[c * 128:(c + 1) * 128, :])
                wo_sb.append(t)
            fhat_sb = const_pool.tile([128, 8, 2, 2, 128], F16, tag="fhat")
            nc.sync.dma_start(fhat_sb[:], fhat_d[:])
            bo_sb = const_pool.tile([1, D], F16, tag="bo")
            nc.sync.dma_start(bo_sb[:], bo_d[:])
            ones_sb = const_pool.tile([1, 128], F16, tag="ones")
            nc.sync.dma_start(ones_sb[:], ones_d[:])

            # ---- software-pipelined main loop over half-blocks of 256 rows ----
            # stage A(hb): DMA xT, Q-proj, attn logits + softmax -> pT via XBAR
            # stage B(hb): ohat, final, store — emitted one hb late so the PE
            # never waits on freshly-computed softmax results.
            p_state = {}
            pts_state = {}

            pending_xposes = []

            def flush_xposes(cur_idx=None):
                # XBAR: per-128-col-block transpose of (n, [hh,k]) ->
                # [k-part, (hh, kchunk), n] written into the s half.
                # Each batch is issued one full stage after production (age
                # >= 2 half-stages) so the dependency (normalize) is long
                # satisfied -> the in-order DMA queue never spins on it,
                # keeping x-loads behind it on schedule.
                while pending_xposes and (
                    cur_idx is None or pending_xposes[0][0] <= cur_idx - 2
                ):
                    _, ptsg, s, e_g = pending_xposes.pop(0)
                    nc.sync.dma_start_transpose(ptsg[:, :, :, s, :], e_g[:])

            def stage_a(hb, s_range):
                blk = hb // 2
                if 0 in s_range:
                    # prefetch next block's x tiles first: dep-free loads at
                    # the head of the DMA queue, ~2 stages before consumption
                    nblk = blk + (hb % 2)
                    if nblk + 1 < NHB // 2 and (nblk + 1, "qt") not in p_state:
                        if nblk + 1 not in xt_state:
                            load_x(nblk + 1)
                flush_xposes(2 * hb + s_range[0])

                def q_chunks(b, cos):
                    # compute Q chunks `cos` of block b
                    if b not in xt_state:
                        load_x(b)
                    xt = xt_state[b]
                    qt = p_state.setdefault((b, "qt"), {})
                    for co in cos:
                        qp = qfpsum.tile([128, 512], F32, tag="qf", name=f"qp{co}")
                        for ck in range(8):
                            nc.tensor.matmul(
                                qp[:],
                                wq_sb[ck][:, co * 128:(co + 1) * 128],
                                xt[ck][:],
                                start=(ck == 0),
                                stop=(ck == 7),
                            )
                        q_sb = qt_pool.tile([128, 512], F32R, tag="qt", name=f"q{co}")
                        nc.scalar.copy(q_sb[:], qp[:])
                        qt[co] = q_sb
                    if max(cos) == 7:
                        xt_state.pop(b, None)

                if 0 in s_range:
                    if hb == 0:
                        q_chunks(0, range(8))
                    elif hb % 2 == 0:
                        q_chunks(blk, range(4, 8))
                    else:
                        if blk + 1 < NHB // 2:
                            q_chunks(blk + 1, range(0, 4))
                qt = p_state[(blk, "qt")]

                for s in s_range:
                    sb = (hb % 2) * 2 + s
                    for gp in range(2):  # group-pairs of 4 pairs = 8 heads
                        aps = []
                        negmax = stat_pool.tile([128, 8], F32, tag="negmax")
                        ssum = stat_pool.tile([128, 8], F16, tag="ssum")
                        for jj in range(4):
                            j = 4 * gp + jj
                            ap_ = apsum.tile([128, 2 * KP], F32, tag="ap", name=f"ap{j}")
                            nc.tensor.matmul(
                                ap_[:],
                                qt[j][:, sb * 128:(sb + 1) * 128],
                                ehat_sb[:, j, :],
                                start=True,
                                stop=True,
                            )
                            aps.append(ap_)
                            nc.vector.reduce_max(
                                negmax[:, 2 * jj:2 * jj + 2],
                                ap_[:].rearrange("p (c k) -> p c k", c=2),
                                axis=mybir.AxisListType.X, negate=True,
                            )
                        e_g = e_pool.tile([128, 8, KP], F16, tag="eg", name=f"eg{gp}")
                        for hh in range(8):
                            nc.scalar.activation(
                                e_g[:, hh, :],
                                aps[hh // 2][:, (hh % 2) * KP:(hh % 2 + 1) * KP],
                                mybir.ActivationFunctionType.Exp,
                                bias=negmax[:, hh:hh + 1],
                            )
                        # fp16 ssum store keeps the reduce + normalize on the
                        # DVE 2x 16-bit path; the reduce accumulates
                        # internally at full precision, only the store rounds
                        with nc.allow_low_precision(reason="fp16 softmax-sum store"):
                            nc.vector.reduce_sum(
                                ssum[:], e_g[:], axis=mybir.AxisListType.X
                            )
                        recip = stat_pool.tile([128, 8], F32, tag="recip")
                        nc.vector.reciprocal(recip[:], ssum[:])
                        # duplicated fp16 recip: gives the broadcast operand a
                        # packed stride-1 innermost dim (DVE 2x requirement)
                        recip16 = stat_pool.tile([128, 8, 2], F16, tag="recip16")
                        nc.vector.tensor_copy(
                            recip16[:], recip[:].unsqueeze(2).broadcast_to((128, 8, 2))
                        )
                        nc.vector.tensor_tensor(
                            e_g[:].rearrange("p h (kk kl) -> p h kk kl", kl=2),
                            e_g[:].rearrange("p h (kk kl) -> p h kk kl", kl=2),
                            recip16[:].unsqueeze(2).broadcast_to((128, 8, KP // 2, 2)),
                            op=mybir.AluOpType.mult,
                        )
                        if s == 0:
                            ptsg = pt_pool.tile(
                                [128, 8, 2, 2, 128], F16, tag="ptsg", name=f"pts{gp}"
                            )
                            pts_state[(hb, gp)] = ptsg
                        else:
                            ptsg = pts_state[(hb, gp)]
                        pending_xposes.append((2 * hb + s, ptsg, s, e_g))

            def stage_b(hb):
                r0 = hb * HBLK
                ptsg = [pts_state.pop((hb, gp)) for gp in range(2)]
                ohatT = []
                for j in range(8):
                    op_ = opsum.tile([128, HBLK], F32, tag="op", name=f"op{j}")
                    first = True
                    for hh2 in range(2):
                        gp, hh = j // 4, (j % 4) * 2
                        for c in range(2):
                            nc.tensor.matmul(
                                op_[:],
                                fhat_sb[:, j, hh2, c, :],
                                ptsg[gp][:, hh + hh2, c, :, :],
                                start=first,
                                stop=(hh2 == 1 and c == 1),
                            )
                            first = False
                    oT = ohat_pool.tile([128, HBLK], F16, tag="ohatT", name=f"oT{j}")
                    nc.scalar.copy(oT[:], op_[:])
                    ohatT.append(oT)
                for s in range(2):
                    for half in range(2):
                        fp_ = qfpsum.tile([128, 512], F32, tag="qf", name=f"fp{s}{half}")
                        for j in range(8):
                            nc.tensor.matmul(
                                fp_[:],
                                ohatT[j][:, s * 128:(s + 1) * 128],
                                wo_sb[j][:, half * 512:(half + 1) * 512],
                                start=(j == 0),
                                stop=False,
                            )
                        nc.tensor.matmul(
                            fp_[:],
                            ones_sb[:],
                            bo_sb[0:1, half * 512:(half + 1) * 512],
                            start=False,
                            stop=True,
                        )
                        o_sb = out_pool.tile([128, 512], F16, tag="osb", name=f"o{s}{half}")
                        nc.scalar.copy(o_sb[:], fp_[:])
                        # store via the Activation HWDGE queue: keeps the sync
                        # queue spin-free for x-loads + XBAR transposes
                        nc.scalar.dma_start(
                            out_d[r0 + s * 128:r0 + (s + 1) * 128,
                                  half * 512:(half + 1) * 512],
                            o_sb[:],
                        )

            # stage_b consumes the XBAR-transposed attention two half-blocks
            # late (DMA transpose latency never on the PE's path), and is
            # emitted between the s=0 and s=1 softmax halves so the PE chews
            # stage_b matmuls while the s=0 softmax chain drains on
            # scalar/DVE (frees the logits PSUM banks for s=1)
            for hb in range(NHB + 2):
                if hb < NHB:
                    stage_a(hb, (0,))
                else:
                    flush_xposes()
                if hb >= 2:
                    stage_b(hb - 2)
                if hb < NHB:
                    stage_a(hb, (1,))
                    if hb % 2 == 1:
                        p_state.pop((hb // 2, "qt"), None)

    nc.compile()
    return nc


def _prep_inputs(x, Wq, Wk, Wv, E, F, Wo, bo):
    x = np.asarray(x, dtype=np.float32)
    Wq = np.asarray(Wq, dtype=np.float32)
    Wk = np.asarray(Wk, dtype=np.float32)
    Wv = np.asarray(Wv, dtype=np.float32)
    E = np.asarray(E, dtype=np.float32)
    F_ = np.asarray(F, dtype=np.float32)
    Wo = np.asarray(Wo, dtype=np.float32)
    bo = np.asarray(bo, dtype=np.float32)

    xsum = x.sum(axis=1)                     # (B, D)
    S_k = xsum @ Wk.T                        # (B, D)
    S_v = xsum @ Wv.T                        # (B, D)

    wqT = np.ascontiguousarray(Wq.T).astype(np.float16)
    woT = np.ascontiguousarray(Wo.T).astype(np.float16)
    bo_row = bo.reshape(1, D).astype(np.float16)

    in_maps = []
    for core in range(NCORES):
        b, half = core // 2, core % 2
        xs = x[b, half * NH:(half + 1) * NH, :]          # (NH, D)
        xT = np.ascontiguousarray(xs.T).astype(np.float16)  # (D, NH)

        # E-hat: block-diagonal per head pair -> one (128,512) rhs per pair
        ehat = np.zeros((128, 8, 2 * KP), dtype=np.float32)
        for h in range(H):
            sk = S_k[b, h * HD:(h + 1) * HD]             # (64,)
            j, hh = h // 2, h % 2
            ehat[hh * 64:hh * 64 + 64, j, hh * KP:(hh + 1) * KP] = (E.T * sk[:, None]) / 8.0
        ehat = _round_fp32r(ehat)

        # F-hat: block-diagonal pair packing, (128, pair, head-in-pair, chunk, 64*2)
        fhat = np.zeros((128, 8, 2, 2, 128), dtype=np.float32)
        for h in range(H):
            sv = S_v[b, h * HD:(h + 1) * HD]             # (64,)
            fh = F_ * sv[None, :]                        # (KP, 64)
            j, hh = h // 2, h % 2
            for c in range(2):
                fhat[:, j, hh, c, hh * 64:(hh + 1) * 64] = fh[c * 128:(c + 1) * 128, :]
        fhat = fhat.astype(np.float16)

        in_maps.append({
            "xT": xT, "wqT": wqT, "woT": woT, "ehat": ehat,
            "fhat": fhat, "bo": bo_row,
            "ones": np.ones((1, 128), dtype=np.float16),
        })
    return in_maps


def _run(inputs: dict, trace: bool = False, tmpdir: str | None = None):
    if "nc" not in _CACHE:
        _CACHE["nc"] = _build()
    nc = _CACHE["nc"]
    in_maps = _prep_inputs(**inputs)
    res = bass_utils.run_bass_kernel_spmd(
        nc, in_maps, core_ids=list(range(NCORES)), trace=trace, tmpdir=tmpdir
    )
    out = np.empty((B, N, D), dtype=np.float32)
    for core in range(NCORES):
        b, half = core // 2, core % 2
        out[b, half * NH:(half + 1) * NH, :] = res.results[core]["out"]
    return out, res


def kernel(**inputs) -> np.ndarray:
    out, _ = _run(inputs)
    return out
